# revision 23
# baseline (speedup 1.0000x reference)
"""Trainium2 Bass kernel for nn_DebedderNeuronGroup_index.

Math (per layer l, with kn=KN[l], ksci=KS[l]*CI[l], i_dim=ksci+1):
    out[b, k, o] = sum_d x[b, off_l + k, d] * W_l[o, d] + b_l[o]
    y[b, S_l + k*ksci + o] = out[b, k, o]          for o <  ksci
    y[b, S_l + kn*ksci + k] = out[b, k, ksci]      (bias column tail block)
The five layers' outputs exactly tile y's 1,422,218 columns.

Strategy: pure data parallelism over batch (16 per core, 8 cores).
Host pre-transposes x to xT[d, token] (token order layer-major then
batch-major) and W to WT[d, o], both bf16. Per 128-token subtile the
tokens sit on PSUM partitions (stationary operand = xT tile), o on the
free dim, so HBM stores are [tokens, o] tiles whose rows form contiguous
runs in y.  The o=ksci bias column is folded into the main matmul (o-tiles
split i_dim evenly, <=512 wide); its drained values are staged per layer
as [128, n_subtiles] and turned into token-major rows by PE transposes at
the layer end, giving contiguous tail-block stores.  For kn=256 layers the
stage columns are written half-interleaved so two transposes of contiguous
halves yield y_col's [batch, 256] layout directly.

PSUM drains (+bias, f32 -> f16) are the #2 resource after the PE; they
rotate over three paths so no single engine saturates:
  V  : DVE tensor_add(psum, bias)             (f32-rate on DVE)
  AD : ACT copy psum->ob, DVE 16-bit += bias  (2x rate on DVE)
  AG : ACT copy psum->ob, GpSimd 16-bit += bias
Ring usage: sync = x loads + large stores, scalar = drain copies only,
gpsimd = table loads + small/irregular stores + stage copies + adds.

Layer order [2, 3, 0, 4, 1]: L2 starts fast (1.2 MB table, split over
two DMA rings) and its 118 us of dense matmuls hide all remaining table
traffic; L3 streams its 4.2 MB table + 32 MB of stores through the long
middle; L0/L4's tiny strided stores hide under compute instead of
clogging the startup or the tail; the kernel ends on L1 whose final
store is only 0.15 MB.
"""

import numpy as np
import ml_dtypes

import concourse.bass as bass
import concourse.mybir as mybir
from concourse import bacc
from concourse.tile import TileContext
from concourse.bass_utils import run_bass_kernel_spmd

# ---------------------------------------------------------------- constants
N_CORES = 8
B = 128
BPC = B // N_CORES            # batches per core = 16
D = 512
KN = [64, 128, 256, 256, 10]
KSCI = [27, 576, 1152, 4096, 256]
IDIM = [k + 1 for k in KSCI]
START = [0, 1792, 75648, 370816, 1419648]
I_TOTAL = 1422218
TOK = sum(KN)                 # 714 tokens per batch
TOKL = [BPC * k for k in KN]  # tokens per core per layer
XOFF = np.cumsum([0] + TOKL).tolist()   # token offset per layer in xT
NTOK = XOFF[-1]               # 11424
BBOFF = np.cumsum([0] + IDIM).tolist()  # bias offset per layer (incl col)
BBTOT = BBOFF[-1]             # 6112
TLOAD = 1024                  # tokens per x DMA chunk
BF16 = mybir.dt.bfloat16
F16 = mybir.dt.float16
F32 = mybir.dt.float32


# even o-tile split of i_dim (each tile <= 512 to fit one PSUM bank)
def _osplit(idim):
    nt = -(-idim // 512)
    base, rem = divmod(idim, nt)
    sizes = [base + 1] * rem + [base] * (nt - rem)
    offs = np.cumsum([0] + sizes).tolist()
    return [(offs[i], sizes[i]) for i in range(nt)]


OSPLIT = [_osplit(i) for i in IDIM]

# token-subtile width per layer (whole batches when kn < 128)
TS = [128 if kn >= 128 else (128 // kn) * kn for kn in KN]
NSUB = [-(-TOKL[l] // TS[l]) for l in range(5)]   # [8, 16, 32, 32, 2]

SEQ = [2, 3, 0, 4, 1]

# drain-path rotation: V = DVE direct add; AD = ACT copy + DVE add;
# AG = ACT copy + GpSimd add.
PAT = ["V", "AD", "V", "AG", "V", "AD", "V", "AG", "V", "AD"]

_cache = {}
last_results = None


def _build_bass():
    nc = bacc.Bacc(
        "TRN2", target_bir_lowering=False, debug=False, num_devices=N_CORES
    )
    xT = nc.declare_dram_parameter("xT", [D, NTOK], BF16, isOutput=False)
    WT = [
        nc.declare_dram_parameter(f"WT{l}", [D, IDIM[l]], BF16, isOutput=False)
        for l in range(5)
    ]
    BB = nc.declare_dram_parameter("BB", [128, BBTOT], F16, isOutput=False)
    IDN = nc.declare_dram_parameter("IDN", [128, 128], BF16, isOutput=False)
    y = nc.declare_dram_parameter("y", [BPC, I_TOTAL], F16, isOutput=True)

    xT3 = xT[:, :].rearrange("(c p) t -> p c t", p=128)      # [128, 4, NTOK]

    with TileContext(nc) as tc:
        with (
            tc.tile_pool(name="wt", bufs=1) as wt_pool,
            tc.tile_pool(name="bias", bufs=1) as bias_pool,
            tc.tile_pool(name="x", bufs=4) as x_pool,
            tc.tile_pool(name="out", bufs=3) as out_pool,
            tc.tile_pool(name="stg", bufs=1) as stg_pool,
            tc.tile_pool(name="tcol", bufs=1) as tcol_pool,
            tc.tile_pool(name="ps", bufs=7, space="PSUM") as ps_pool,
            tc.tile_pool(name="pst", bufs=1, space="PSUM") as pst_pool,
        ):
            bb = bias_pool.tile([128, BBTOT], F16, tag="bb")
            idn = bias_pool.tile([128, 128], BF16, tag="idn")

            wt_tiles = {
                l: wt_pool.tile(
                    [128, 4 * IDIM[l]], BF16, tag=f"wt{l}", name=f"wt{l}"
                )
                for l in range(5)
            }

            def emit_table_load(l, engs):
                t3 = wt_tiles[l][:].rearrange("p (c o) -> p c o", c=4)
                wsrc = WT[l][:, :].rearrange("(c p) o -> p c o", p=128)
                ne = len(engs)
                for i, eng in enumerate(engs):
                    c0, c1 = i * 4 // ne, (i + 1) * 4 // ne
                    eng.dma_start(out=t3[:, c0:c1, :], in_=wsrc[:, c0:c1, :])
                engs[0].dma_start(
                    out=bb[:, BBOFF[l] : BBOFF[l] + IDIM[l]],
                    in_=BB[:, BBOFF[l] : BBOFF[l] + IDIM[l]],
                )
                return t3

            # Only the first layer's table loads upfront (split across the
            # SWDGE and ACT rings for the fastest start).  The rest are
            # emitted mid-stream (see INJECT below) so the early DMA fabric
            # isn't saturated: table traffic then hides under compute.
            # x and the main stores stream on the SP ring.
            wt3_by_layer = {}
            wt3_by_layer[2] = emit_table_load(2, [nc.gpsimd, nc.scalar])
            nc.gpsimd.dma_start(out=idn[:], in_=IDN[:, :])

            # (layer-index-in-SEQ, chunk-index) -> deferred table loads,
            # emitted just before that chunk's work (i.e. after the
            # previous chunks' matmuls are queued).
            INJECT = {
                (0, 2): [(3, [nc.gpsimd, nc.scalar])],
                (1, 1): [(0, [nc.gpsimd]), (4, [nc.gpsimd])],
                (1, 3): [(1, [nc.gpsimd])],
            }

            ot_counter = 0
            for li, l in enumerate(SEQ):
                wt3_l = wt_tiles[l][:].rearrange("p (c o) -> p c o", c=4)
                kn, ksci, idim = KN[l], KSCI[l], IDIM[l]
                ts, n_sub = TS[l], NSUB[l]
                half = kn == 256     # two stage halves (2 subtiles per batch)
                y_main = y[:, START[l] : START[l] + kn * ksci].rearrange(
                    "b (k o) -> b k o", o=ksci
                )
                y_col = y[:, START[l] + kn * ksci : START[l] + kn * ksci + kn]

                stage = stg_pool.tile([128, n_sub], BF16, tag=f"stg{l}")
                if TOKL[l] % ts:
                    # last subtile is short: zero the unwritten stage rows
                    nc.gpsimd.memset(stage[:], 0.0)

                for ci, t0 in enumerate(range(0, TOKL[l], TLOAD)):
                    for dl, dengs in INJECT.get((li, ci), []):
                        emit_table_load(dl, dengs)
                    tl = min(TLOAD, TOKL[l] - t0)
                    xt = x_pool.tile([128, 4 * TLOAD], BF16, tag="xt")
                    xt3 = xt[:].rearrange("p (c t) -> p c t", c=4)
                    nc.sync.dma_start(
                        out=xt3[:, :, :tl],
                        in_=xT3[:, :, XOFF[l] + t0 : XOFF[l] + t0 + tl],
                    )
                    ob = None
                    for s0 in range(0, tl, ts):
                        sl = min(ts, tl - s0)          # tokens in subtile
                        tok = t0 + s0                  # layer-token index
                        si = tok // ts                 # subtile index
                        # kn>=128 layers pair two subtiles into one ob so
                        # each store covers 256 contiguous k-rows (one DMA,
                        # half the store/semaphore traffic)
                        hh = si % 2 if kn >= 128 else 0
                        if ob is None or hh == 0:
                            ob = out_pool.tile([128, 2 * 4097], F16, tag="ob")
                        oboff = hh * idim
                        for o0, no in OSPLIT[l]:
                            ps = ps_pool.tile([128, 512], F32, tag="ps")
                            for dc in range(4):
                                nc.tensor.matmul(
                                    out=ps[:sl, :no],
                                    lhsT=xt3[:, dc, s0 : s0 + sl],
                                    rhs=wt3_l[:, dc, o0 : o0 + no],
                                    start=(dc == 0),
                                    stop=(dc == 3),
                                )
                            obs = ob[:sl, oboff + o0 : oboff + o0 + no]
                            bbs = bb[:sl, BBOFF[l] + o0 : BBOFF[l] + o0 + no]
                            # tiny layers, the pipeline-priming first chunk,
                            # and the kernel's final subtiles take the
                            # low-latency single-hop DVE path
                            if (
                                l in (0, 4)
                                or (li == 0 and ci == 0)
                                or (l == 1 and si >= n_sub - 2)
                            ):
                                path = "V"
                            else:
                                path = PAT[ot_counter % len(PAT)]
                            ot_counter += 1
                            if path == "V":
                                nc.vector.tensor_add(
                                    out=obs, in0=ps[:sl, :no], in1=bbs
                                )
                            else:
                                nc.scalar.copy(out=obs, in_=ps[:sl, :no])
                                eng = nc.vector if path == "AD" else nc.gpsimd
                                eng.tensor_add(out=obs, in0=obs, in1=bbs)
                        # stage the bias column for the layer-end transpose
                        scol = (si // 2 + (si % 2) * (n_sub // 2)) if half else si
                        nc.gpsimd.tensor_copy(
                            out=stage[:sl, scol : scol + 1],
                            in_=ob[:sl, oboff + ksci : oboff + ksci + 1],
                        )
                        # main store: k-rows are contiguous runs in y
                        b0 = tok // kn
                        if kn < 128:
                            nb = sl // kn
                            for bi in range(nb):
                                nc.gpsimd.dma_start(
                                    out=y_main[b0 + bi, :, :],
                                    in_=ob[bi * kn : (bi + 1) * kn, :ksci],
                                )
                        elif hh == 1:
                            # paired store: [128p, 2h, ksci] on both sides
                            src = ob[:, : 2 * idim].rearrange(
                                "p (h o) -> p h o", o=idim
                            )[:, :, :ksci]
                            if kn == 256:   # pair = one batch (k halves)
                                dst = y_main[b0, :, :].rearrange(
                                    "(h k) o -> k h o", h=2
                                )
                            else:           # kn == 128: pair = two batches
                                dst = y_main[b0 - 1 : b0 + 1, :, :].rearrange(
                                    "b k o -> k b o"
                                )
                            nc.sync.dma_start(out=dst, in_=src)

                # ---- layer end: transpose staged bias column to token-major
                pst = pst_pool.tile([128, 256], BF16, tag="pst")
                tcol = tcol_pool.tile([32, 256], F16, tag=f"tc{l}")
                if half:
                    # stage halves -> pst[:16, :256] == y_col[batch, 256]
                    h = n_sub // 2
                    for j in range(2):
                        nc.tensor.transpose(
                            out=pst[:h, j * 128 : (j + 1) * 128],
                            in_=stage[:, j * h : (j + 1) * h],
                            identity=idn[:, :],
                        )
                    nc.vector.tensor_copy(
                        out=tcol[:h, :256], in_=pst[:h, :256]
                    )
                    nc.sync.dma_start(out=y_col[:, :], in_=tcol[:h, :256])
                else:
                    nc.tensor.transpose(
                        out=pst[:n_sub, :128],
                        in_=stage[:, :],
                        identity=idn[:, :],
                    )
                    nc.vector.tensor_copy(
                        out=tcol[:n_sub, :128], in_=pst[:n_sub, :128]
                    )
                    if l == 0:   # row s = batches (2s, 2s+1), 64 tokens each
                        nc.gpsimd.dma_start(
                            out=y_col[:, :].rearrange("(s b) f -> s b f", b=2),
                            in_=tcol[:n_sub, :128].rearrange(
                                "s (b f) -> s b f", b=2
                            ),
                        )
                    elif l == 1:  # row s = batch s
                        nc.gpsimd.dma_start(
                            out=y_col[:, :], in_=tcol[:n_sub, :128]
                        )
                    else:         # l == 4, ts=120: row 0 = b 0-11, row 1 = 12-15
                        nc.gpsimd.dma_start(
                            out=y_col[0:12, :].rearrange("(r b) f -> r b f", r=1),
                            in_=tcol[0:1, :120].rearrange(
                                "r (b f) -> r b f", b=12
                            ),
                        )
                        nc.gpsimd.dma_start(
                            out=y_col[12:16, :].rearrange("(r b) f -> r b f", r=1),
                            in_=tcol[1:2, :40].rearrange(
                                "r (b f) -> r b f", b=4
                            ),
                        )
    nc.compile()
    return nc


def _prep_inputs(inputs):
    x = np.asarray(inputs["x"], dtype=np.float32)
    xb = x.astype(ml_dtypes.bfloat16)
    in_maps = []
    shared = {}
    for l in range(5):
        W = np.asarray(inputs[f"W{l}"], dtype=np.float32)
        shared[f"WT{l}"] = np.ascontiguousarray(W.astype(ml_dtypes.bfloat16).T)
    bbvec = np.concatenate(
        [np.asarray(inputs[f"b{l}"], dtype=np.float32) for l in range(5)]
    )
    shared["BB"] = np.ascontiguousarray(
        np.broadcast_to(bbvec.astype(np.float16), (128, BBTOT))
    )
    shared["IDN"] = np.eye(128, dtype=ml_dtypes.bfloat16)
    off = np.cumsum([0] + KN).tolist()
    for c in range(N_CORES):
        xc = xb[c * BPC : (c + 1) * BPC]  # [16, 714, 512] bf16
        parts = [
            np.transpose(xc[:, off[l] : off[l] + KN[l]], (2, 0, 1)).reshape(D, -1)
            for l in range(5)
        ]
        xT = np.ascontiguousarray(np.concatenate(parts, axis=1))  # [512, 11424]
        in_maps.append({"xT": xT, **shared})
    return in_maps


def kernel(**inputs):
    global last_results
    if "nc" not in _cache:
        _cache["nc"] = _build_bass()
    nc = _cache["nc"]
    in_maps = _prep_inputs(inputs)
    res = run_bass_kernel_spmd(nc, in_maps, list(range(N_CORES)))
    last_results = res
    y = np.concatenate(
        [res.results[c]["y"].astype(np.float32) for c in range(N_CORES)], axis=0
    )
    return y


# revision 25
# speedup vs baseline: 1.0081x; 1.0081x over previous
"""Trainium2 Bass kernel for nn_DebedderNeuronGroup_index.

Math (per layer l, with kn=KN[l], ksci=KS[l]*CI[l], i_dim=ksci+1):
    out[b, k, o] = sum_d x[b, off_l + k, d] * W_l[o, d] + b_l[o]
    y[b, S_l + k*ksci + o] = out[b, k, o]          for o <  ksci
    y[b, S_l + kn*ksci + k] = out[b, k, ksci]      (bias column tail block)
The five layers' outputs exactly tile y's 1,422,218 columns.

Strategy: pure data parallelism over batch (16 per core, 8 cores).
Host pre-transposes x to xT[d, token] (token order layer-major then
batch-major) and W to WT[d, o], both bf16. Per 128-token subtile the
tokens sit on PSUM partitions (stationary operand = xT tile), o on the
free dim, so HBM stores are [tokens, o] tiles whose rows form contiguous
runs in y.  The o=ksci bias column is folded into the main matmul (o-tiles
split i_dim evenly, <=512 wide); its drained values are staged per layer
as [128, n_subtiles] and turned into token-major rows by PE transposes at
the layer end, giving contiguous tail-block stores.  For kn=256 layers the
stage columns are written half-interleaved so two transposes of contiguous
halves yield y_col's [batch, 256] layout directly.

PSUM drains (+bias, f32 -> f16) are the #2 resource after the PE; they
rotate over three paths so no single engine saturates:
  V  : DVE tensor_add(psum, bias)             (f32-rate on DVE)
  AD : ACT copy psum->ob, DVE 16-bit += bias  (2x rate on DVE)
  AG : ACT copy psum->ob, GpSimd 16-bit += bias
Ring usage: sync = x loads + large stores, scalar = drain copies only,
gpsimd = table loads + small/irregular stores + stage copies + adds.

Layer order [2, 3, 0, 4, 1]: L2 starts fast (1.2 MB table, split over
two DMA rings) and its 118 us of dense matmuls hide all remaining table
traffic; L3 streams its 4.2 MB table + 32 MB of stores through the long
middle; L0/L4's tiny strided stores hide under compute instead of
clogging the startup or the tail; the kernel ends on L1 whose final
store is only 0.15 MB.
"""

import numpy as np
import ml_dtypes

import concourse.bass as bass
import concourse.mybir as mybir
from concourse import bacc
from concourse.tile import TileContext
from concourse.bass_utils import run_bass_kernel_spmd

# ---------------------------------------------------------------- constants
N_CORES = 8
B = 128
BPC = B // N_CORES            # batches per core = 16
D = 512
KN = [64, 128, 256, 256, 10]
KSCI = [27, 576, 1152, 4096, 256]
IDIM = [k + 1 for k in KSCI]
START = [0, 1792, 75648, 370816, 1419648]
I_TOTAL = 1422218
TOK = sum(KN)                 # 714 tokens per batch
TOKL = [BPC * k for k in KN]  # tokens per core per layer
XOFF = np.cumsum([0] + TOKL).tolist()   # token offset per layer in xT
NTOK = XOFF[-1]               # 11424
BBOFF = np.cumsum([0] + IDIM).tolist()  # bias offset per layer (incl col)
BBTOT = BBOFF[-1]             # 6112
TLOAD = 1024                  # tokens per x DMA chunk
BF16 = mybir.dt.bfloat16
F16 = mybir.dt.float16
F32 = mybir.dt.float32


# even o-tile split of i_dim (each tile <= 512 to fit one PSUM bank)
def _osplit(idim):
    nt = -(-idim // 512)
    base, rem = divmod(idim, nt)
    sizes = [base + 1] * rem + [base] * (nt - rem)
    offs = np.cumsum([0] + sizes).tolist()
    return [(offs[i], sizes[i]) for i in range(nt)]


OSPLIT = [_osplit(i) for i in IDIM]

# token-subtile width per layer (whole batches when kn < 128)
TS = [128 if kn >= 128 else (128 // kn) * kn for kn in KN]
NSUB = [-(-TOKL[l] // TS[l]) for l in range(5)]   # [8, 16, 32, 32, 2]

SEQ = [2, 3, 0, 4, 1]

# drain-path rotation: V = DVE direct add; AD = ACT copy + DVE add;
# AG = ACT copy + GpSimd add.
PAT = ["V", "AD", "V", "AG", "V", "AD", "V", "AG", "V", "AD"]

_cache = {}
last_results = None


def _build_bass():
    nc = bacc.Bacc(
        "TRN2", target_bir_lowering=False, debug=False, num_devices=N_CORES
    )
    xT = nc.declare_dram_parameter("xT", [D, NTOK], BF16, isOutput=False)
    WT = [
        nc.declare_dram_parameter(f"WT{l}", [D, IDIM[l]], BF16, isOutput=False)
        for l in range(5)
    ]
    BB = nc.declare_dram_parameter("BB", [128, BBTOT], F16, isOutput=False)
    IDN = nc.declare_dram_parameter("IDN", [128, 128], BF16, isOutput=False)
    y = nc.declare_dram_parameter("y", [BPC, I_TOTAL], F16, isOutput=True)

    xT3 = xT[:, :].rearrange("(c p) t -> p c t", p=128)      # [128, 4, NTOK]

    with TileContext(nc) as tc:
        with (
            tc.tile_pool(name="wt", bufs=1) as wt_pool,
            tc.tile_pool(name="bias", bufs=1) as bias_pool,
            tc.tile_pool(name="x", bufs=4) as x_pool,
            tc.tile_pool(name="out", bufs=4) as out_pool,
            tc.tile_pool(name="stg", bufs=1) as stg_pool,
            tc.tile_pool(name="tcol", bufs=1) as tcol_pool,
            tc.tile_pool(name="ps", bufs=7, space="PSUM") as ps_pool,
            tc.tile_pool(name="pst", bufs=1, space="PSUM") as pst_pool,
        ):
            bb = bias_pool.tile([128, BBTOT], F16, tag="bb")
            idn = bias_pool.tile([128, 128], BF16, tag="idn")

            wt_tiles = {
                l: wt_pool.tile(
                    [128, 4 * IDIM[l]], BF16, tag=f"wt{l}", name=f"wt{l}"
                )
                for l in range(5)
            }

            def emit_table_load(l, engs):
                t3 = wt_tiles[l][:].rearrange("p (c o) -> p c o", c=4)
                wsrc = WT[l][:, :].rearrange("(c p) o -> p c o", p=128)
                ne = len(engs)
                for i, eng in enumerate(engs):
                    c0, c1 = i * 4 // ne, (i + 1) * 4 // ne
                    eng.dma_start(out=t3[:, c0:c1, :], in_=wsrc[:, c0:c1, :])
                engs[0].dma_start(
                    out=bb[:, BBOFF[l] : BBOFF[l] + IDIM[l]],
                    in_=BB[:, BBOFF[l] : BBOFF[l] + IDIM[l]],
                )
                return t3

            # The two big tables (L2 then L3) load upfront, split across
            # the SWDGE and ACT rings; they finish inside the store-free
            # first ~25 us (chunk 0 drains buffer in ob).  SDMA shares
            # bandwidth round-robin per ring, so the table rings must be
            # IDLE once the store stream ramps — the remaining small
            # tables are emitted mid-L3 (INJECT) where their ~1 MB is
            # absorbed.  x and the main stores stream on the SP ring.
            wt3_by_layer = {}
            wt3_by_layer[2] = emit_table_load(2, [nc.gpsimd, nc.scalar])
            nc.gpsimd.dma_start(out=idn[:], in_=IDN[:, :])
            wt3_by_layer[3] = emit_table_load(3, [nc.gpsimd, nc.scalar])

            # (layer-index-in-SEQ, chunk-index) -> deferred table loads,
            # emitted just before that chunk's work (i.e. after the
            # previous chunks' matmuls are queued).
            INJECT = {
                (1, 1): [(0, [nc.gpsimd]), (4, [nc.gpsimd])],
                (1, 3): [(1, [nc.gpsimd])],
            }

            ot_counter = 0
            for li, l in enumerate(SEQ):
                wt3_l = wt_tiles[l][:].rearrange("p (c o) -> p c o", c=4)
                kn, ksci, idim = KN[l], KSCI[l], IDIM[l]
                ts, n_sub = TS[l], NSUB[l]
                half = kn == 256     # two stage halves (2 subtiles per batch)
                y_main = y[:, START[l] : START[l] + kn * ksci].rearrange(
                    "b (k o) -> b k o", o=ksci
                )
                y_col = y[:, START[l] + kn * ksci : START[l] + kn * ksci + kn]

                stage = stg_pool.tile([128, n_sub], BF16, tag=f"stg{l}")
                if TOKL[l] % ts:
                    # last subtile is short: zero the unwritten stage rows
                    nc.gpsimd.memset(stage[:], 0.0)

                for ci, t0 in enumerate(range(0, TOKL[l], TLOAD)):
                    for dl, dengs in INJECT.get((li, ci), []):
                        emit_table_load(dl, dengs)
                    tl = min(TLOAD, TOKL[l] - t0)
                    xt = x_pool.tile([128, 4 * TLOAD], BF16, tag="xt")
                    xt3 = xt[:].rearrange("p (c t) -> p c t", c=4)
                    nc.sync.dma_start(
                        out=xt3[:, :, :tl],
                        in_=xT3[:, :, XOFF[l] + t0 : XOFF[l] + t0 + tl],
                    )
                    ob = None
                    for s0 in range(0, tl, ts):
                        sl = min(ts, tl - s0)          # tokens in subtile
                        tok = t0 + s0                  # layer-token index
                        si = tok // ts                 # subtile index
                        # kn>=128 layers pair two subtiles into one ob so
                        # each store covers 256 contiguous k-rows (one DMA,
                        # half the store/semaphore traffic)
                        hh = si % 2 if kn >= 128 else 0
                        if ob is None or hh == 0:
                            ob = out_pool.tile([128, 2 * 4097], F16, tag="ob")
                        oboff = hh * idim
                        for o0, no in OSPLIT[l]:
                            ps = ps_pool.tile([128, 512], F32, tag="ps")
                            for dc in range(4):
                                nc.tensor.matmul(
                                    out=ps[:sl, :no],
                                    lhsT=xt3[:, dc, s0 : s0 + sl],
                                    rhs=wt3_l[:, dc, o0 : o0 + no],
                                    start=(dc == 0),
                                    stop=(dc == 3),
                                )
                            obs = ob[:sl, oboff + o0 : oboff + o0 + no]
                            bbs = bb[:sl, BBOFF[l] + o0 : BBOFF[l] + o0 + no]
                            # tiny layers, the pipeline-priming first chunk,
                            # and the kernel's final subtiles take the
                            # low-latency single-hop DVE path
                            if (
                                l in (0, 4)
                                or (li == 0 and ci == 0)
                                or (l == 1 and si >= n_sub - 2)
                            ):
                                path = "V"
                            else:
                                path = PAT[ot_counter % len(PAT)]
                            ot_counter += 1
                            if path == "V":
                                nc.vector.tensor_add(
                                    out=obs, in0=ps[:sl, :no], in1=bbs
                                )
                            else:
                                nc.scalar.copy(out=obs, in_=ps[:sl, :no])
                                eng = nc.vector if path == "AD" else nc.gpsimd
                                eng.tensor_add(out=obs, in0=obs, in1=bbs)
                        # stage the bias column for the layer-end transpose
                        scol = (si // 2 + (si % 2) * (n_sub // 2)) if half else si
                        nc.gpsimd.tensor_copy(
                            out=stage[:sl, scol : scol + 1],
                            in_=ob[:sl, oboff + ksci : oboff + ksci + 1],
                        )
                        # main store: k-rows are contiguous runs in y
                        b0 = tok // kn
                        if kn < 128:
                            nb = sl // kn
                            for bi in range(nb):
                                nc.gpsimd.dma_start(
                                    out=y_main[b0 + bi, :, :],
                                    in_=ob[bi * kn : (bi + 1) * kn, :ksci],
                                )
                        elif hh == 1:
                            # paired store: [128p, 2h, ksci] on both sides
                            src = ob[:, : 2 * idim].rearrange(
                                "p (h o) -> p h o", o=idim
                            )[:, :, :ksci]
                            if kn == 256:   # pair = one batch (k halves)
                                dst = y_main[b0, :, :].rearrange(
                                    "(h k) o -> k h o", h=2
                                )
                            else:           # kn == 128: pair = two batches
                                dst = y_main[b0 - 1 : b0 + 1, :, :].rearrange(
                                    "b k o -> k b o"
                                )
                            nc.sync.dma_start(out=dst, in_=src)

                # ---- layer end: transpose staged bias column to token-major
                pst = pst_pool.tile([128, 256], BF16, tag="pst")
                tcol = tcol_pool.tile([32, 256], F16, tag=f"tc{l}")
                if half:
                    # stage halves -> pst[:16, :256] == y_col[batch, 256]
                    h = n_sub // 2
                    for j in range(2):
                        nc.tensor.transpose(
                            out=pst[:h, j * 128 : (j + 1) * 128],
                            in_=stage[:, j * h : (j + 1) * h],
                            identity=idn[:, :],
                        )
                    nc.vector.tensor_copy(
                        out=tcol[:h, :256], in_=pst[:h, :256]
                    )
                    nc.sync.dma_start(out=y_col[:, :], in_=tcol[:h, :256])
                else:
                    nc.tensor.transpose(
                        out=pst[:n_sub, :128],
                        in_=stage[:, :],
                        identity=idn[:, :],
                    )
                    nc.vector.tensor_copy(
                        out=tcol[:n_sub, :128], in_=pst[:n_sub, :128]
                    )
                    if l == 0:   # row s = batches (2s, 2s+1), 64 tokens each
                        nc.gpsimd.dma_start(
                            out=y_col[:, :].rearrange("(s b) f -> s b f", b=2),
                            in_=tcol[:n_sub, :128].rearrange(
                                "s (b f) -> s b f", b=2
                            ),
                        )
                    elif l == 1:  # row s = batch s
                        nc.gpsimd.dma_start(
                            out=y_col[:, :], in_=tcol[:n_sub, :128]
                        )
                    else:         # l == 4, ts=120: row 0 = b 0-11, row 1 = 12-15
                        nc.gpsimd.dma_start(
                            out=y_col[0:12, :].rearrange("(r b) f -> r b f", r=1),
                            in_=tcol[0:1, :120].rearrange(
                                "r (b f) -> r b f", b=12
                            ),
                        )
                        nc.gpsimd.dma_start(
                            out=y_col[12:16, :].rearrange("(r b) f -> r b f", r=1),
                            in_=tcol[1:2, :40].rearrange(
                                "r (b f) -> r b f", b=4
                            ),
                        )
    nc.compile()
    return nc


def _prep_inputs(inputs):
    x = np.asarray(inputs["x"], dtype=np.float32)
    xb = x.astype(ml_dtypes.bfloat16)
    in_maps = []
    shared = {}
    for l in range(5):
        W = np.asarray(inputs[f"W{l}"], dtype=np.float32)
        shared[f"WT{l}"] = np.ascontiguousarray(W.astype(ml_dtypes.bfloat16).T)
    bbvec = np.concatenate(
        [np.asarray(inputs[f"b{l}"], dtype=np.float32) for l in range(5)]
    )
    shared["BB"] = np.ascontiguousarray(
        np.broadcast_to(bbvec.astype(np.float16), (128, BBTOT))
    )
    shared["IDN"] = np.eye(128, dtype=ml_dtypes.bfloat16)
    off = np.cumsum([0] + KN).tolist()
    for c in range(N_CORES):
        xc = xb[c * BPC : (c + 1) * BPC]  # [16, 714, 512] bf16
        parts = [
            np.transpose(xc[:, off[l] : off[l] + KN[l]], (2, 0, 1)).reshape(D, -1)
            for l in range(5)
        ]
        xT = np.ascontiguousarray(np.concatenate(parts, axis=1))  # [512, 11424]
        in_maps.append({"xT": xT, **shared})
    return in_maps


def kernel(**inputs):
    global last_results
    if "nc" not in _cache:
        _cache["nc"] = _build_bass()
    nc = _cache["nc"]
    in_maps = _prep_inputs(inputs)
    res = run_bass_kernel_spmd(nc, in_maps, list(range(N_CORES)))
    last_results = res
    y = np.concatenate(
        [res.results[c]["y"].astype(np.float32) for c in range(N_CORES)], axis=0
    )
    return y


# revision 30
# speedup vs baseline: 1.0163x; 1.0081x over previous
"""Trainium2 Bass kernel for nn_DebedderNeuronGroup_index.

Math (per layer l, with kn=KN[l], ksci=KS[l]*CI[l], i_dim=ksci+1):
    out[b, k, o] = sum_d x[b, off_l + k, d] * W_l[o, d] + b_l[o]
    y[b, S_l + k*ksci + o] = out[b, k, o]          for o <  ksci
    y[b, S_l + kn*ksci + k] = out[b, k, ksci]      (bias column tail block)
The five layers' outputs exactly tile y's 1,422,218 columns.

Strategy: pure data parallelism over batch (16 per core, 8 cores).
Host pre-transposes x to xT[d, token] (token order layer-major then
batch-major) and W to WT[d, o], both bf16. Per 128-token subtile the
tokens sit on PSUM partitions (stationary operand = xT tile), o on the
free dim, so HBM stores are [tokens, o] tiles whose rows form contiguous
runs in y.  The o=ksci bias column is folded into the main matmul (o-tiles
split i_dim evenly, <=512 wide); its drained values are staged per layer
as [128, n_subtiles] and turned into token-major rows by PE transposes at
the layer end, giving contiguous tail-block stores.  For kn=256 layers the
stage columns are written half-interleaved so two transposes of contiguous
halves yield y_col's [batch, 256] layout directly.

PSUM drains (+bias, f32 -> f16) are the #2 resource after the PE; they
rotate over three paths so no single engine saturates:
  V  : DVE tensor_add(psum, bias)             (f32-rate on DVE)
  AD : ACT copy psum->ob, DVE 16-bit += bias  (2x rate on DVE)
  AG : ACT copy psum->ob, GpSimd 16-bit += bias
Ring usage: sync = x loads + large stores, scalar = drain copies only,
gpsimd = table loads + small/irregular stores + stage copies + adds.

Layer order [2, 3, 0, 4, 1]: L2 starts fast (1.2 MB table, split over
two DMA rings) and its 118 us of dense matmuls hide all remaining table
traffic; L3 streams its 4.2 MB table + 32 MB of stores through the long
middle; L0/L4's tiny strided stores hide under compute instead of
clogging the startup or the tail; the kernel ends on L1 whose final
store is only 0.15 MB.
"""

import numpy as np
import ml_dtypes

import concourse.bass as bass
import concourse.mybir as mybir
from concourse import bacc
from concourse.tile import TileContext
from concourse.bass_utils import run_bass_kernel_spmd

# ---------------------------------------------------------------- constants
N_CORES = 8
B = 128
BPC = B // N_CORES            # batches per core = 16
D = 512
KN = [64, 128, 256, 256, 10]
KSCI = [27, 576, 1152, 4096, 256]
IDIM = [k + 1 for k in KSCI]
START = [0, 1792, 75648, 370816, 1419648]
I_TOTAL = 1422218
TOK = sum(KN)                 # 714 tokens per batch
TOKL = [BPC * k for k in KN]  # tokens per core per layer
XOFF = np.cumsum([0] + TOKL).tolist()   # token offset per layer in xT
NTOK = XOFF[-1]               # 11424
BBOFF = np.cumsum([0] + IDIM).tolist()  # bias offset per layer (incl col)
BBTOT = BBOFF[-1]             # 6112
TLOAD = 1024                  # tokens per x DMA chunk
BF16 = mybir.dt.bfloat16
F16 = mybir.dt.float16
F32 = mybir.dt.float32


# even o-tile split of i_dim (each tile <= 512 to fit one PSUM bank)
def _osplit(idim):
    nt = -(-idim // 512)
    base, rem = divmod(idim, nt)
    sizes = [base + 1] * rem + [base] * (nt - rem)
    offs = np.cumsum([0] + sizes).tolist()
    return [(offs[i], sizes[i]) for i in range(nt)]


OSPLIT = [_osplit(i) for i in IDIM]

# token-subtile width per layer (whole batches when kn < 128)
TS = [128 if kn >= 128 else (128 // kn) * kn for kn in KN]
NSUB = [-(-TOKL[l] // TS[l]) for l in range(5)]   # [8, 16, 32, 32, 2]

SEQ = [0, 4, 2, 3, 1]

# drain-path rotation: V = DVE direct add; AD = ACT copy + DVE add;
# AG = ACT copy + GpSimd add.
PAT = ["V", "AD", "V", "AG", "V", "AD", "V", "AG", "V", "AD"]

_cache = {}
last_results = None


def _build_bass():
    nc = bacc.Bacc(
        "TRN2", target_bir_lowering=False, debug=False, num_devices=N_CORES
    )
    xT = nc.declare_dram_parameter("xT", [D, NTOK], BF16, isOutput=False)
    WT = [
        nc.declare_dram_parameter(f"WT{l}", [D, IDIM[l]], BF16, isOutput=False)
        for l in range(5)
    ]
    BB = nc.declare_dram_parameter("BB", [128, BBTOT], F16, isOutput=False)
    IDN = nc.declare_dram_parameter("IDN", [128, 128], BF16, isOutput=False)
    y = nc.declare_dram_parameter("y", [BPC, I_TOTAL], F16, isOutput=True)

    xT3 = xT[:, :].rearrange("(c p) t -> p c t", p=128)      # [128, 4, NTOK]

    with TileContext(nc) as tc:
        with (
            tc.tile_pool(name="wt", bufs=1) as wt_pool,
            tc.tile_pool(name="bias", bufs=1) as bias_pool,
            tc.tile_pool(name="x", bufs=4) as x_pool,
            tc.tile_pool(name="out", bufs=4) as out_pool,
            tc.tile_pool(name="out0", bufs=8) as out0_pool,
            tc.tile_pool(name="out4", bufs=2) as out4_pool,
            tc.tile_pool(name="stg", bufs=1) as stg_pool,
            tc.tile_pool(name="tcol", bufs=1) as tcol_pool,
            tc.tile_pool(name="ps", bufs=7, space="PSUM") as ps_pool,
            tc.tile_pool(name="pst", bufs=1, space="PSUM") as pst_pool,
        ):
            bb = bias_pool.tile([128, BBTOT], F16, tag="bb")
            idn = bias_pool.tile([128, 128], BF16, tag="idn")

            wt_tiles = {
                l: wt_pool.tile(
                    [128, 4 * IDIM[l]], BF16, tag=f"wt{l}", name=f"wt{l}"
                )
                for l in range(5)
            }

            def emit_table_load(l, engs):
                t3 = wt_tiles[l][:].rearrange("p (c o) -> p c o", c=4)
                wsrc = WT[l][:, :].rearrange("(c p) o -> p c o", p=128)
                ne = len(engs)
                for i, eng in enumerate(engs):
                    c0, c1 = i * 4 // ne, (i + 1) * 4 // ne
                    eng.dma_start(out=t3[:, c0:c1, :], in_=wsrc[:, c0:c1, :])
                engs[0].dma_start(
                    out=bb[:, BBOFF[l] : BBOFF[l] + IDIM[l]],
                    in_=BB[:, BBOFF[l] : BBOFF[l] + IDIM[l]],
                )
                return t3

            # L0/L4's tiny tables plus the two big ones (L2 then L3) load
            # upfront, split across the SWDGE and ACT rings; they finish
            # inside the store-free first ~25 us (L0/L4 run from their own
            # output pools so nothing waits on their slow stores, and L2
            # chunk 0's drains buffer in ob).  SDMA shares bandwidth
            # round-robin per ring, so the table rings must be IDLE once
            # the store stream ramps — L1's small table is emitted mid-L3
            # (INJECT) where its ~0.6 MB is absorbed.  x and the main
            # stores stream on the SP ring.
            emit_table_load(0, [nc.gpsimd])
            emit_table_load(4, [nc.gpsimd])
            emit_table_load(2, [nc.gpsimd, nc.scalar])
            nc.gpsimd.dma_start(out=idn[:], in_=IDN[:, :])
            emit_table_load(3, [nc.gpsimd, nc.scalar])

            # (layer-index-in-SEQ, chunk-index) -> deferred table loads,
            # emitted just before that chunk's work (i.e. after the
            # previous chunks' matmuls are queued).
            INJECT = {
                (3, 1): [(1, [nc.gpsimd])],
            }

            ot_counter = 0
            for li, l in enumerate(SEQ):
                wt3_l = wt_tiles[l][:].rearrange("p (c o) -> p c o", c=4)
                kn, ksci, idim = KN[l], KSCI[l], IDIM[l]
                ts, n_sub = TS[l], NSUB[l]
                half = kn == 256     # two stage halves (2 subtiles per batch)
                y_main = y[:, START[l] : START[l] + kn * ksci].rearrange(
                    "b (k o) -> b k o", o=ksci
                )
                y_col = y[:, START[l] + kn * ksci : START[l] + kn * ksci + kn]

                stage = stg_pool.tile([128, n_sub], BF16, tag=f"stg{l}")
                if TOKL[l] % ts:
                    # last subtile is short: zero the unwritten stage rows
                    nc.gpsimd.memset(stage[:], 0.0)

                for ci, t0 in enumerate(range(0, TOKL[l], TLOAD)):
                    for dl, dengs in INJECT.get((li, ci), []):
                        emit_table_load(dl, dengs)
                    tl = min(TLOAD, TOKL[l] - t0)
                    xt = x_pool.tile([128, 4 * TLOAD], BF16, tag="xt")
                    xt3 = xt[:].rearrange("p (c t) -> p c t", c=4)
                    nc.sync.dma_start(
                        out=xt3[:, :, :tl],
                        in_=xT3[:, :, XOFF[l] + t0 : XOFF[l] + t0 + tl],
                    )
                    ob = None
                    for s0 in range(0, tl, ts):
                        sl = min(ts, tl - s0)          # tokens in subtile
                        tok = t0 + s0                  # layer-token index
                        si = tok // ts                 # subtile index
                        # kn>=128 layers pair two subtiles into one ob so
                        # each store covers 256 contiguous k-rows (one DMA,
                        # half the store/semaphore traffic)
                        hh = si % 2 if kn >= 128 else 0
                        if l == 0:
                            ob = out0_pool.tile([128, 28], F16, tag="ob0")
                        elif l == 4:
                            ob = out4_pool.tile([128, 257], F16, tag="ob4")
                        elif ob is None or hh == 0:
                            ob = out_pool.tile([128, 2 * 4097], F16, tag="ob")
                        oboff = hh * idim
                        for o0, no in OSPLIT[l]:
                            ps = ps_pool.tile([128, 512], F32, tag="ps")
                            for dc in range(4):
                                nc.tensor.matmul(
                                    out=ps[:sl, :no],
                                    lhsT=xt3[:, dc, s0 : s0 + sl],
                                    rhs=wt3_l[:, dc, o0 : o0 + no],
                                    start=(dc == 0),
                                    stop=(dc == 3),
                                )
                            obs = ob[:sl, oboff + o0 : oboff + o0 + no]
                            bbs = bb[:sl, BBOFF[l] + o0 : BBOFF[l] + o0 + no]
                            # tiny layers, the pipeline-priming first chunk,
                            # and the kernel's final subtiles take the
                            # low-latency single-hop DVE path
                            if (
                                l in (0, 4)
                                or (li == 2 and ci == 0)
                                or (l == 1 and si >= n_sub - 2)
                            ):
                                path = "V"
                            else:
                                path = PAT[ot_counter % len(PAT)]
                            ot_counter += 1
                            if path == "V":
                                nc.vector.tensor_add(
                                    out=obs, in0=ps[:sl, :no], in1=bbs
                                )
                            else:
                                nc.scalar.copy(out=obs, in_=ps[:sl, :no])
                                eng = nc.vector if path == "AD" else nc.gpsimd
                                eng.tensor_add(out=obs, in0=obs, in1=bbs)
                        # stage the bias column for the layer-end transpose
                        scol = (si // 2 + (si % 2) * (n_sub // 2)) if half else si
                        nc.gpsimd.tensor_copy(
                            out=stage[:sl, scol : scol + 1],
                            in_=ob[:sl, oboff + ksci : oboff + ksci + 1],
                        )
                        # main store: k-rows are contiguous runs in y
                        b0 = tok // kn
                        if kn < 128:
                            nb = sl // kn
                            for bi in range(nb):
                                nc.gpsimd.dma_start(
                                    out=y_main[b0 + bi, :, :],
                                    in_=ob[bi * kn : (bi + 1) * kn, :ksci],
                                )
                        elif hh == 1:
                            # paired store: [128p, 2h, ksci] on both sides
                            src = ob[:, : 2 * idim].rearrange(
                                "p (h o) -> p h o", o=idim
                            )[:, :, :ksci]
                            if kn == 256:   # pair = one batch (k halves)
                                dst = y_main[b0, :, :].rearrange(
                                    "(h k) o -> k h o", h=2
                                )
                            else:           # kn == 128: pair = two batches
                                dst = y_main[b0 - 1 : b0 + 1, :, :].rearrange(
                                    "b k o -> k b o"
                                )
                            nc.sync.dma_start(out=dst, in_=src)

                # ---- layer end: transpose staged bias column to token-major
                pst = pst_pool.tile([128, 256], BF16, tag="pst")
                tcol = tcol_pool.tile([32, 256], F16, tag=f"tc{l}")
                if half:
                    # stage halves -> pst[:16, :256] == y_col[batch, 256]
                    h = n_sub // 2
                    for j in range(2):
                        nc.tensor.transpose(
                            out=pst[:h, j * 128 : (j + 1) * 128],
                            in_=stage[:, j * h : (j + 1) * h],
                            identity=idn[:, :],
                        )
                    nc.vector.tensor_copy(
                        out=tcol[:h, :256], in_=pst[:h, :256]
                    )
                    nc.sync.dma_start(out=y_col[:, :], in_=tcol[:h, :256])
                else:
                    nc.tensor.transpose(
                        out=pst[:n_sub, :128],
                        in_=stage[:, :],
                        identity=idn[:, :],
                    )
                    nc.vector.tensor_copy(
                        out=tcol[:n_sub, :128], in_=pst[:n_sub, :128]
                    )
                    if l == 0:   # row s = batches (2s, 2s+1), 64 tokens each
                        nc.gpsimd.dma_start(
                            out=y_col[:, :].rearrange("(s b) f -> s b f", b=2),
                            in_=tcol[:n_sub, :128].rearrange(
                                "s (b f) -> s b f", b=2
                            ),
                        )
                    elif l == 1:  # row s = batch s
                        nc.gpsimd.dma_start(
                            out=y_col[:, :], in_=tcol[:n_sub, :128]
                        )
                    else:         # l == 4, ts=120: row 0 = b 0-11, row 1 = 12-15
                        nc.gpsimd.dma_start(
                            out=y_col[0:12, :].rearrange("(r b) f -> r b f", r=1),
                            in_=tcol[0:1, :120].rearrange(
                                "r (b f) -> r b f", b=12
                            ),
                        )
                        nc.gpsimd.dma_start(
                            out=y_col[12:16, :].rearrange("(r b) f -> r b f", r=1),
                            in_=tcol[1:2, :40].rearrange(
                                "r (b f) -> r b f", b=4
                            ),
                        )
    nc.compile()
    return nc


def _prep_inputs(inputs):
    x = np.asarray(inputs["x"], dtype=np.float32)
    xb = x.astype(ml_dtypes.bfloat16)
    in_maps = []
    shared = {}
    for l in range(5):
        W = np.asarray(inputs[f"W{l}"], dtype=np.float32)
        shared[f"WT{l}"] = np.ascontiguousarray(W.astype(ml_dtypes.bfloat16).T)
    bbvec = np.concatenate(
        [np.asarray(inputs[f"b{l}"], dtype=np.float32) for l in range(5)]
    )
    shared["BB"] = np.ascontiguousarray(
        np.broadcast_to(bbvec.astype(np.float16), (128, BBTOT))
    )
    shared["IDN"] = np.eye(128, dtype=ml_dtypes.bfloat16)
    off = np.cumsum([0] + KN).tolist()
    for c in range(N_CORES):
        xc = xb[c * BPC : (c + 1) * BPC]  # [16, 714, 512] bf16
        parts = [
            np.transpose(xc[:, off[l] : off[l] + KN[l]], (2, 0, 1)).reshape(D, -1)
            for l in range(5)
        ]
        xT = np.ascontiguousarray(np.concatenate(parts, axis=1))  # [512, 11424]
        in_maps.append({"xT": xT, **shared})
    return in_maps


def kernel(**inputs):
    global last_results
    if "nc" not in _cache:
        _cache["nc"] = _build_bass()
    nc = _cache["nc"]
    in_maps = _prep_inputs(inputs)
    res = run_bass_kernel_spmd(nc, in_maps, list(range(N_CORES)))
    last_results = res
    y = np.concatenate(
        [res.results[c]["y"].astype(np.float32) for c in range(N_CORES)], axis=0
    )
    return y
